# revision 24
# baseline (speedup 1.0000x reference)
"""Gemma3 sliding-window attention on 8 TRN2 NeuronCores (Bass/Tile).

Sharding: core c -> batch b=c//4, head-group g=c%4 (q-heads 2g,2g+1,
kv-head g). Per core: QKV projections in transposed [hd, seq] layout,
per-head RMSNorm via ones-matmul row-reduce + K=1 broadcast matmul,
RoPE with host-folded (1+w) tables, exp-softmax without max-subtraction
(window-bounded scores), PV, one 8-core AllGather of attention outputs,
then full-wo o-proj of the core's seq quarter. The rank-dependent read
of the gathered buffer uses an indirect (index tensor) DMA with
host-computed per-core indices.
"""
import os
import numpy as np
import ml_dtypes
import jax

import concourse.bass as bass
import concourse.mybir as mybir
from concourse import bacc
from concourse.tile import TileContext
from concourse.bass2jax import _bass_exec_p, install_neuronx_cc_hook
from jax.sharding import Mesh, PartitionSpec, NamedSharding
from jax.experimental.shard_map import shard_map

F32 = mybir.dt.float32
BF16 = mybir.dt.bfloat16

B, S, HID = 2, 2048, 2560
NH, NKV, HD = 8, 4, 256
WIN = 1024
NCORES = 8
EPS = 1e-6
SCALE = HD ** -0.5

SB = 512                  # seq block
NBLK = S // SB            # 4
KTH = HID // 128          # 20 hid k-tiles
ST = S // 128             # 16 seq tiles
QCOLS = 2 * HD            # 512 q cols per core
KCOLS = HD                # 256 kv cols per core


def _ktiles(n):
    """k-tiles for q-block n: list of (t, j0, j1) sub-block spans."""
    out = []
    for t in range(max(0, 4 * n - 8), 4 * n + 4):
        j0 = max(0, t - 4 * n)
        j1 = min(3, t - 4 * n + 8)
        out.append((t, j0, j1))
    return out


def _norm_rope(nc, tmp, ps1, psb, psL, psU, cl, sl, cu, su, destL, destU,
               span, cns):
    """RMSNorm + RoPE on a transposed [hd,seq] head; psL/psU [128,SB] f32 psum.

    destL = (psL*R)*cl - (psU*R)*sl ; destU = (psU*R)*cu + (psL*R)*su
    where R = broadcast(1/sqrt(mean+eps)); (1+w) (and 1/16 for q) folded
    into the cl/sl/cu/su tables host-side.
    """
    ones_col, ones_row, eps_col = cns
    sqL = tmp.tile([128, SB], BF16, tag="sq", name="sqL")
    nc.scalar.activation(sqL, psL, mybir.ActivationFunctionType.Square)
    sqU = tmp.tile([128, SB], BF16, tag="sq", name="sqU")
    nc.scalar.activation(sqU, psU, mybir.ActivationFunctionType.Square)
    # copy raw q to SBUF right away so the PSUM banks free early; the
    # norm-multiply chain then runs SBUF-only
    qrL = tmp.tile([128, SB], BF16, tag="qn", name="qrL", bufs=4)
    nc.vector.tensor_copy(qrL, psL)
    qrU = tmp.tile([128, SB], BF16, tag="qn", name="qrU", bufs=4)
    nc.vector.tensor_copy(qrU, psU)
    ssq = ps1.tile([1, SB], F32, tag="ps1", name="ssq")
    nc.tensor.matmul(ssq, ones_col, sqL, start=True, stop=False)
    nc.tensor.matmul(ssq, ones_col, sqU, start=False, stop=True)
    ssb = tmp.tile([1, SB], F32, tag="row", name="ssb")
    nc.scalar.activation(ssb, ssq, mybir.ActivationFunctionType.Sqrt,
                         bias=eps_col[:1], scale=1.0 / HD)
    inv = tmp.tile([1, SB], F32, tag="row", name="inv")
    nc.vector.reciprocal(inv, ssb)
    r_ps = psb.tile([128, SB], F32, tag="ps", name="r_ps")
    nc.tensor.matmul(r_ps, ones_row, inv, start=True, stop=True)
    r_sb = tmp.tile([128, SB], F32, tag="rsb", name="r_sb")
    nc.scalar.activation(r_sb, r_ps, mybir.ActivationFunctionType.Copy)
    qnL = tmp.tile([128, SB], BF16, tag="qn", name="qnL", bufs=4)
    nc.vector.tensor_mul(qnL, qrL, r_sb)
    qnU = tmp.tile([128, SB], BF16, tag="qn", name="qnU", bufs=4)
    nc.vector.tensor_mul(qnU, qrU, r_sb)
    tA = tmp.tile([128, SB], BF16, tag="rope", name="tA")
    nc.vector.tensor_mul(tA, qnL, cl[:, span])
    tB = tmp.tile([128, SB], BF16, tag="rope", name="tB")
    nc.vector.tensor_mul(tB, qnU, sl[:, span])
    nc.vector.tensor_sub(destL, tA, tB)
    tC = tmp.tile([128, SB], BF16, tag="rope", name="tC")
    nc.vector.tensor_mul(tC, qnU, cu[:, span])
    tD = tmp.tile([128, SB], BF16, tag="rope", name="tD")
    nc.vector.tensor_mul(tD, qnL, su[:, span])
    nc.vector.tensor_add(destU, tC, tD)


def build_program(debug=False, stage=99):
    stage = int(os.environ.get("KSTAGE", stage))
    krep = int(os.environ.get("KREP", "1"))
    assert not (debug and krep > 1)
    nc = bacc.Bacc("TRN2", target_bir_lowering=False, debug=False,
                   num_devices=NCORES)
    xT = nc.dram_tensor("xT", [KTH, 128, S], BF16, kind="ExternalInput")
    wq = nc.dram_tensor("wq", [KTH, 128, QCOLS], BF16, kind="ExternalInput")
    wk = nc.dram_tensor("wk", [KTH, 128, KCOLS], BF16, kind="ExternalInput")
    wv = nc.dram_tensor("wv", [KTH, 128, KCOLS], BF16, kind="ExternalInput")
    wo = nc.dram_tensor("wo", [ST, 128, HID], BF16, kind="ExternalInput")
    ropes_in = {}
    for nm in ("cql", "sql", "cqu", "squ", "ckl", "skl", "cku", "sku"):
        ropes_in[nm] = nc.dram_tensor(nm, [128, S], BF16, kind="ExternalInput")
    maskD_in = nc.dram_tensor("maskD", [128, 128], BF16, kind="ExternalInput")
    maskE_in = nc.dram_tensor("maskE", [128, 128], BF16, kind="ExternalInput")
    gidx_in = nc.dram_tensor("gidx", [128, ST], mybir.dt.int32,
                             kind="ExternalInput")
    out = nc.dram_tensor("out", [SB, HID], F32, kind="ExternalOutput")
    if debug:
        dbg_qT = nc.dram_tensor("dbg_qT", [4, 128, S], F32,
                                kind="ExternalOutput")
        dbg_kT = nc.dram_tensor("dbg_kT", [2, 128, S], F32,
                                kind="ExternalOutput")
        dbg_v = nc.dram_tensor("dbg_v", [ST, 128, KCOLS], F32,
                               kind="ExternalOutput")
        dbg_oT = nc.dram_tensor("dbg_oT", [NBLK, 4, 128, SB], F32,
                                kind="ExternalOutput")

    with TileContext(nc) as tc:
        with (
            tc.tile_pool(name="consts", bufs=1) as consts_p,
            tc.tile_pool(name="persist", bufs=1) as persist,
            tc.tile_pool(name="xstream", bufs=2) as xpool,
            tc.tile_pool(name="tmp", bufs=2) as tmp,
            tc.tile_pool(name="pb", bufs=6) as pbp,
            tc.tile_pool(name="ob", bufs=2) as obp,
            tc.tile_pool(name="wop", bufs=10) as wop,
            tc.tile_pool(name="outp", bufs=2) as outp,
            tc.tile_pool(name="ps", bufs=6, space="PSUM") as psb,
            tc.tile_pool(name="ps1", bufs=2, space="PSUM") as ps1,
            tc.tile_pool(name="dram", bufs=2, space="DRAM") as dram,
        ):
            # ---- constants ----
            # per-ktile loads, interleaved with the first x block so the
            # first projection matmuls start as soon as slice 0 lands
            wq_sb = consts_p.tile([128, KTH, QCOLS], BF16)
            wk_sb = consts_p.tile([128, KTH, KCOLS], BF16)
            wv_sb = consts_p.tile([128, KTH, KCOLS], BF16)
            x_t0 = xpool.tile([128, KTH, SB], BF16, tag="x", name="x_first")
            for kt in range(KTH):
                nc.sync.dma_start(wq_sb[:, kt], wq.ap()[kt])
                nc.sync.dma_start(x_t0[:, kt], xT.ap()[kt, :, 0:SB])
                nc.sync.dma_start(wk_sb[:, kt], wk.ap()[kt])
                nc.sync.dma_start(wv_sb[:, kt], wv.ap()[kt])
            ropes = {}
            for nm, t_in in ropes_in.items():
                r = consts_p.tile([128, S], BF16, name=f"rope_{nm}",
                                  tag=f"rope_{nm}")
                nc.sync.dma_start(r, t_in.ap())
                ropes[nm] = r
            maskD = consts_p.tile([128, 128], BF16)
            nc.sync.dma_start(maskD, maskD_in.ap())
            maskE = consts_p.tile([128, 128], BF16)
            nc.sync.dma_start(maskE, maskE_in.ap())
            gidx_sb = consts_p.tile([128, ST], mybir.dt.int32)
            nc.sync.dma_start(gidx_sb, gidx_in.ap())
            ones_col = consts_p.tile([128, 1], BF16)
            nc.vector.memset(ones_col, 1.0)
            ones_row = consts_p.tile([1, 128], F32)
            nc.vector.memset(ones_row, 1.0)
            eps_col = consts_p.tile([128, 1], F32)
            nc.vector.memset(eps_col, EPS)
            cns = (ones_col, ones_row, eps_col)

            # ---- persistent activations ----
            qT = persist.tile([128, 4, S], BF16)
            kT = persist.tile([128, 2, S], BF16)
            v_sb = persist.tile([128, ST, KCOLS], BF16)
            oTg = persist.tile([128, ST, SB], BF16)

            for rep in range(krep):
                # ============ phase A: projections ============
                for n in range(NBLK):
                    span = slice(n * SB, (n + 1) * SB)
                    if rep == 0 and n == 0:
                        x_t = x_t0
                    else:
                        x_t = xpool.tile([128, KTH, SB], BF16, tag="x",
                                         name=f"x_{rep}_{n}")
                        for kt in range(KTH):
                            nc.sync.dma_start(x_t[:, kt],
                                              xT.ap()[kt, :, span])
                    units = [
                        (wq_sb, 0, "q", qT, 0),
                        (wq_sb, 2, "q", qT, 2),
                        (wk_sb, 0, "k", kT, 0),
                    ]
                    for w_sb, mb, kind, dstT, dm in units:
                        psL = psb.tile([128, SB], F32, tag="ps",
                                       name=f"pp{rep}_{n}_{kind}{mb}L")
                        psU = psb.tile([128, SB], F32, tag="ps",
                                       name=f"pp{rep}_{n}_{kind}{mb}U")
                        for kt in range(KTH):
                            nc.tensor.matmul(
                                psL, w_sb[:, kt, mb * 128:(mb + 1) * 128],
                                x_t[:, kt], start=(kt == 0),
                                stop=(kt == KTH - 1))
                        for kt in range(KTH):
                            nc.tensor.matmul(
                                psU, w_sb[:, kt, (mb + 1) * 128:(mb + 2) * 128],
                                x_t[:, kt], start=(kt == 0),
                                stop=(kt == KTH - 1))
                        pre = "cq" if kind == "q" else "ck"
                        prs = "sq" if kind == "q" else "sk"
                        _norm_rope(nc, tmp, ps1, psb, psL, psU,
                                   ropes[pre + "l"], ropes[prs + "l"],
                                   ropes[pre + "u"], ropes[prs + "u"],
                                   dstT[:, dm, span], dstT[:, dm + 1, span],
                                   span, cns)
                    for st in range(4):
                        ps = psb.tile([128, SB], F32, tag="ps",
                                      name=f"vp{rep}_{n}_{st}")
                        for kt in range(KTH):
                            nc.tensor.matmul(
                                ps[:, :KCOLS],
                                x_t[:, kt, st * 128:(st + 1) * 128],
                                wv_sb[:, kt], start=(kt == 0),
                                stop=(kt == KTH - 1))
                        nc.vector.tensor_copy(v_sb[:, n * 4 + st],
                                              ps[:, :KCOLS])

                if debug:
                    for m in range(4):
                        for nn in range(NBLK):
                            d = tmp.tile([128, SB], F32, tag="dbg",
                                         name=f"dq{m}_{nn}")
                            nc.vector.tensor_copy(
                                d, qT[:, m, nn * SB:(nn + 1) * SB])
                            nc.sync.dma_start(
                                dbg_qT.ap()[m, :, nn * SB:(nn + 1) * SB], d)
                    for m in range(2):
                        for nn in range(NBLK):
                            d = tmp.tile([128, SB], F32, tag="dbg",
                                         name=f"dk{m}_{nn}")
                            nc.vector.tensor_copy(
                                d, kT[:, m, nn * SB:(nn + 1) * SB])
                            nc.sync.dma_start(
                                dbg_kT.ap()[m, :, nn * SB:(nn + 1) * SB], d)
                    for st_i in range(ST):
                        d = tmp.tile([128, KCOLS], F32, tag="dbg",
                                     name=f"dv{st_i}")
                        nc.vector.tensor_copy(d, v_sb[:, st_i])
                        nc.sync.dma_start(dbg_v.ap()[st_i], d)

                if stage < 2:
                    continue

                # ============ phase B: attention ============
                agis = [dram.tile([4 * 128, SB], BF16, bufs=1,
                                  name=f"agi{rep}_{n_}", tag=f"agi{rep}_{n_}")
                        for n_ in range(NBLK)]
                agos = [dram.tile([NCORES * 4 * 128, SB], BF16,
                                  addr_space="Shared", bufs=1,
                                  name=f"ago{rep}_{n_}", tag=f"ago{rep}_{n_}")
                        for n_ in range(NBLK)]
                flat = dram.tile([NBLK * NCORES * 4 * 128, SB], BF16, bufs=1,
                                 name=f"flat{rep}", tag=f"flat{rep}")
                for n in range(NBLK):
                    kts = _ktiles(n)
                    last = len(kts) - 1
                    ob = obp.tile([128, 4, SB], BF16, tag="ob",
                                  name=f"ob{rep}_{n}")
                    pv = [psb.tile([128, SB], F32, tag="ps",
                                   name=f"pv{rep}_{n}_{hv}")
                          for hv in range(4)]
                    den = ps1.tile([33, SB], F32, tag="ps1",
                                   name=f"den{rep}_{n}")
                    for i, (t, j0, j1) in enumerate(kts):
                        qoff = j0 * 128
                        w = (j1 - j0 + 1) * 128
                        qsp = slice(n * SB + qoff, n * SB + qoff + w)
                        for h in range(2):
                            sc = psb.tile([128, SB], F32, tag="ps",
                                          name=f"sc{rep}_{n}_{h}_{t}")
                            nc.tensor.matmul(sc[:, qoff:qoff + w],
                                             kT[:, 0, t * 128:(t + 1) * 128],
                                             qT[:, 2 * h, qsp],
                                             start=True, stop=False)
                            nc.tensor.matmul(sc[:, qoff:qoff + w],
                                             kT[:, 1, t * 128:(t + 1) * 128],
                                             qT[:, 2 * h + 1, qsp],
                                             start=False, stop=True)
                            pb = pbp.tile([128, SB], BF16, tag="pb",
                                          name=f"pb{rep}_{n}_{h}_{t}")
                            if j0 > 0:
                                nc.vector.memset(pb[:, :qoff], 0.0)
                            if j1 < 3:
                                nc.vector.memset(pb[:, qoff + w:], 0.0)
                            nc.scalar.activation(
                                pb[:, qoff:qoff + w], sc[:, qoff:qoff + w],
                                mybir.ActivationFunctionType.Exp)
                            jD = t - 4 * n
                            if 0 <= jD <= 3:
                                sl_ = slice(jD * 128, (jD + 1) * 128)
                                nc.vector.tensor_mul(pb[:, sl_], pb[:, sl_],
                                                     maskD)
                            jE = t - 4 * n + 8
                            if 0 <= jE <= 3:
                                sl_ = slice(jE * 128, (jE + 1) * 128)
                                nc.vector.tensor_mul(pb[:, sl_], pb[:, sl_],
                                                     maskE)
                            nc.tensor.matmul(pv[2 * h], v_sb[:, t, 0:128],
                                             pb, start=(i == 0),
                                             stop=(i == last))
                            nc.tensor.matmul(pv[2 * h + 1],
                                             v_sb[:, t, 128:256], pb,
                                             start=(i == 0), stop=(i == last))
                            nc.tensor.matmul(den[32 * h:32 * h + 1], ones_col,
                                             pb, start=(i == 0),
                                             stop=(i == last))
                    dsb = tmp.tile([33, SB], F32, tag="row",
                                   name=f"dsb{rep}_{n}")
                    nc.scalar.activation(dsb[0:1], den[0:1],
                                         mybir.ActivationFunctionType.Copy)
                    nc.scalar.activation(dsb[32:33], den[32:33],
                                         mybir.ActivationFunctionType.Copy)
                    drec = tmp.tile([33, SB], F32, tag="row",
                                    name=f"drec{rep}_{n}")
                    nc.vector.reciprocal(drec[0:1], dsb[0:1])
                    nc.vector.reciprocal(drec[32:33], dsb[32:33])
                    d1b = tmp.tile([1, SB], F32, tag="row",
                                   name=f"d1b{rep}_{n}")
                    nc.sync.dma_start(d1b, drec[32:33])
                    dsrc = (drec, d1b)
                    for h in range(2):
                        rd_sb = tmp.tile([128, SB], F32, tag="rsb",
                                         name=f"rds{rep}_{n}_{h}")
                        nc.gpsimd.partition_broadcast(
                            rd_sb[:], dsrc[h][0:1])
                        nc.vector.tensor_mul(ob[:, 2 * h + 0], pv[2 * h],
                                             rd_sb)
                        nc.vector.tensor_mul(ob[:, 2 * h + 1], pv[2 * h + 1],
                                             rd_sb)
                    if debug:
                        for m in range(4):
                            d = tmp.tile([128, SB], F32, tag="dbg",
                                         name=f"dob{n}_{m}")
                            nc.vector.tensor_copy(d, ob[:, m])
                            nc.sync.dma_start(dbg_oT.ap()[n, m], d)
                    if stage < 3:
                        continue
                    nc.sync.dma_start(
                        agis[n][:].rearrange("(m p) s -> p m s", p=128), ob)
                    if stage == 6:
                        # sim-only stand-in for the AllGather (local copy)
                        nc.sync.dma_start(agos[n][:4 * 128], agis[n][:])
                    else:
                        nc.gpsimd.collective_compute(
                            "AllGather", mybir.AluOpType.bypass,
                            replica_groups=[list(range(NCORES))],
                            ins=[agis[n][:]], outs=[agos[n][:]])
                    nc.sync.dma_start(
                        flat[n * NCORES * 4 * 128:(n + 1) * NCORES * 4 * 128],
                        agos[n][:])
                ago = flat

                # ============ phase C: o-proj ============
                if stage < 4:
                    nc.vector.memset(oTg, 0.0)
                for t in range(ST if stage >= 4 else 0):
                    nc.gpsimd.indirect_dma_start(
                        out=oTg[:, t], out_offset=None, in_=ago[:],
                        in_offset=bass.IndirectOffsetOnAxis(
                            ap=gidx_sb[:, t:t + 1], axis=0))
                for hb in range(HID // SB if stage >= 5 else 0):
                    ops = [psb.tile([128, SB], F32, tag="ps",
                                    name=f"ops{rep}_{hb}_{k_}")
                           for k_ in range(4)]
                    for oc in range(ST):
                        wo_t = wop.tile([128, SB], BF16, tag="wo",
                                        name=f"wo{rep}_{hb}_{oc}")
                        nc.sync.dma_start(
                            wo_t, wo.ap()[oc, :, hb * SB:(hb + 1) * SB])
                        for st in range(4):
                            nc.tensor.matmul(
                                ops[st], oTg[:, oc, st * 128:(st + 1) * 128],
                                wo_t, start=(oc == 0), stop=(oc == ST - 1))
                    for st in range(4):
                        osb = outp.tile([128, SB], F32, tag="osb",
                                        name=f"osb{rep}_{hb}_{st}")
                        nc.vector.tensor_copy(osb, ops[st])
                        nc.sync.dma_start(
                            out.ap()[st * 128:(st + 1) * 128,
                                     hb * SB:(hb + 1) * SB], osb)
    nc.compile()
    return nc


# ====================== host side ======================

def _prep_core(c, hidden, cos, sin, wq, wk, wv, wo, q_norm_w, k_norm_w):
    b, g = c // 4, c % 4
    bf = ml_dtypes.bfloat16
    x = np.asarray(hidden[b], np.float32)
    xT = np.ascontiguousarray(x.T).astype(bf).reshape(KTH, 128, S)
    wq_c = np.ascontiguousarray(
        wq[:, g * QCOLS:(g + 1) * QCOLS]).astype(bf).reshape(KTH, 128, QCOLS)
    wk_c = np.ascontiguousarray(
        wk[:, g * KCOLS:(g + 1) * KCOLS]).astype(bf).reshape(KTH, 128, KCOLS)
    wv_c = np.ascontiguousarray(
        wv[:, g * KCOLS:(g + 1) * KCOLS]).astype(bf).reshape(KTH, 128, KCOLS)
    wo_c = np.ascontiguousarray(wo).astype(bf).reshape(ST, 128, HID)

    ct = np.ascontiguousarray(cos[b][:, :128].T, np.float32)   # [128, S]
    st_ = np.ascontiguousarray(sin[b][:, :128].T, np.float32)
    w1q = (1.0 + np.asarray(q_norm_w, np.float32)) * SCALE
    w1k = 1.0 + np.asarray(k_norm_w, np.float32)
    r = {
        "cql": ct * w1q[:128, None], "sql": st_ * w1q[128:, None],
        "cqu": ct * w1q[128:, None], "squ": st_ * w1q[:128, None],
        "ckl": ct * w1k[:128, None], "skl": st_ * w1k[128:, None],
        "cku": ct * w1k[128:, None], "sku": st_ * w1k[:128, None],
    }
    r = {k_: v_.astype(bf) for k_, v_ in r.items()}

    ii = np.arange(128)
    maskD = (ii[None, :] >= ii[:, None]).astype(bf)     # [k, q]: q >= k
    maskE = (ii[None, :] < ii[:, None]).astype(bf)      # q < k
    qq = c % 4
    tt = np.arange(ST)
    rr = 4 * b + tt // 4
    gidx = (qq * (NCORES * SB) + rr[None, :] * SB
            + (tt % 4)[None, :] * 128 + ii[:, None]).astype(np.int32)
    return {"xT": xT, "wq": wq_c, "wk": wk_c, "wv": wv_c, "wo": wo_c,
            **r, "maskD": maskD, "maskE": maskE, "gidx": gidx}


class _Exec:
    def __init__(self, nc, n_cores=NCORES):
        install_neuronx_cc_hook()
        self.nc = nc
        self.n_cores = n_cores
        pname = nc.partition_id_tensor.name if nc.partition_id_tensor else None
        in_names, out_names, out_avals, zero_outs = [], [], [], []
        for alloc in nc.m.functions[0].allocations:
            if not isinstance(alloc, mybir.MemoryLocationSet):
                continue
            name = alloc.memorylocations[0].name
            if alloc.kind == "ExternalInput":
                if name != pname:
                    in_names.append(name)
            elif alloc.kind == "ExternalOutput":
                shape = tuple(alloc.tensor_shape)
                dtype = mybir.dt.np(alloc.dtype)
                out_names.append(name)
                out_avals.append(jax.core.ShapedArray(shape, dtype))
                zero_outs.append(np.zeros(shape, dtype))
        self.in_names, self.out_names = in_names, out_names
        self.out_avals, self.zero_outs = out_avals, zero_outs
        n_params, n_outs = len(in_names), len(out_names)
        all_in = in_names + out_names + ([pname] if pname else [])

        def _body(*args):
            operands = list(args)
            if pname is not None:
                from concourse.bass2jax import partition_id_tensor
                operands.append(partition_id_tensor())
            return tuple(_bass_exec_p.bind(
                *operands, out_avals=tuple(out_avals),
                in_names=tuple(all_in), out_names=tuple(out_names),
                lowering_input_output_aliases=(),
                sim_require_finite=True, sim_require_nnan=True, nc=nc))

        devices = jax.devices()[:n_cores]
        self.mesh = Mesh(np.asarray(devices), ("core",))
        in_specs = (PartitionSpec("core"),) * (n_params + n_outs)
        out_specs = (PartitionSpec("core"),) * n_outs
        self.jitted = jax.jit(
            shard_map(_body, mesh=self.mesh, in_specs=in_specs,
                      out_specs=out_specs, check_rep=False),
            keep_unused=True)
        self._zero_dev = None

    def put_inputs(self, in_maps):
        sh = NamedSharding(self.mesh, PartitionSpec("core"))
        args = [jax.device_put(
            np.concatenate([np.asarray(m[n]) for m in in_maps], axis=0), sh)
            for n in self.in_names]
        if self._zero_dev is None:
            self._zero_dev = [jax.device_put(
                np.zeros((self.n_cores * z.shape[0], *z.shape[1:]), z.dtype),
                sh) for z in self.zero_outs]
        return args + self._zero_dev

    def __call__(self, dev_args):
        return self.jitted(*dev_args)

    def results(self, out_arrs):
        res = []
        for c in range(self.n_cores):
            d = {}
            for i, name in enumerate(self.out_names):
                shape = self.out_avals[i].shape
                d[name] = np.asarray(out_arrs[i]).reshape(
                    self.n_cores, *shape)[c]
            res.append(d)
        return res


_CACHE = {}


def _get_exec(debug=False):
    key = (("dbg" if debug else "rel") + os.environ.get("KSTAGE", "")
           + os.environ.get("KREP", ""))
    if key not in _CACHE:
        _CACHE[key] = _Exec(build_program(debug=debug))
    return _CACHE[key]


def prep_in_maps(inputs):
    return [_prep_core(c, inputs["hidden_states"], inputs["cos"],
                       inputs["sin"], inputs["wq"], inputs["wk"],
                       inputs["wv"], inputs["wo"], inputs["q_norm_w"],
                       inputs["k_norm_w"]) for c in range(NCORES)]


def assemble(results):
    full = np.empty((B, S, HID), np.float32)
    for c in range(NCORES):
        b, qq = c // 4, c % 4
        full[b, qq * SB:(qq + 1) * SB] = results[c]["out"]
    return full


def kernel(hidden_states, cos, sin, wq, wk, wv, wo, q_norm_w, k_norm_w):
    ex = _get_exec(debug=False)
    in_maps = prep_in_maps(dict(
        hidden_states=hidden_states, cos=cos, sin=sin, wq=wq, wk=wk, wv=wv,
        wo=wo, q_norm_w=q_norm_w, k_norm_w=k_norm_w))
    args = ex.put_inputs(in_maps)
    outs = ex(args)
    return assemble(ex.results(outs))
